# revision 14
# baseline (speedup 1.0000x reference)
"""Centroid-similarity (ProtoNet softmax) kernel for 8 trn2 NeuronCores.

Math (per reference):
    counts   = sum_n y[n, c]
    cent     = (y^T @ E) / max(counts, 1)          # divide_no_nan
    out      = softmax(-(|e|^2 + |c|^2 - 2 e.c), axis=C)
softmax is invariant to per-row constants, so |e|^2 drops out:
    out      = softmax(2*cross - sq_c), cross = E @ cent^T, sq_c = |cent|^2

Distribution: data-parallel over N. Each core gets an N/8 = 2048-row shard,
computes partial (y^T E | counts) on the tensor engine, AllReduces the
[C, D+1] stats, then computes its own 2048 x C block of logits + softmax.

Key engineering choices (v3):
  - Inputs converted to fp16 on the host and packed per row-chunk as
    [E_chunk | y_chunk] (one dram tensor, 2304B contiguous per partition
    row): halves HBM-in traffic and fp16 matmul runs at full PE rate.
  - A 1-byte dummy AllGather issues before any compute so the runtime's
    one-time collective bootstrap (barrier + channel handshake, ~10-30us)
    overlaps phase A instead of serializing before the stats AllReduce.
  - The stats AllReduce is split in two halves; centroid prep and the
    first half of matmul #2's contraction run while half 2 is in flight.
  - Stats collective stays fp32 and exp stays fp32: fp16/bf16 there cost
    ~1e-2 rel err each (measured), breaching the 2e-2 gate's margin.
  - E^T for matmul #2 is built on the PE (fp16 transposes, 1 cycle/row),
    4 transposes batched per PSUM tile with a single [128,512] copy out,
    ordered j-major so et d-chunks complete in mm2's consumption order.
  - softmax tail is batched per 512-row block: transposes into one PSUM
    tile, row-sums into one [128,4] tile, one reciprocal, one out DMA.
"""

import numpy as np

import concourse.bacc as bacc
import concourse.bass as bass
import concourse.mybir as mybir
import concourse.tile as tile
from concourse import masks
from concourse.bass_utils import run_bass_kernel_spmd
from concourse.tile import add_dep_helper

N, C, D = 16384, 128, 1024
CORES = 8
NS = N // CORES          # 2048 rows per core
P = 128                  # partition dim
NCH = NS // P            # 16 n-chunks per core
DCH = D // P             # 8 d-chunks
NB = NS // 512           # 4 moving-dim blocks for matmul #2
W = D + C                # packed row: [e (1024) | y (128)] fp16
DH = D // 2              # AllReduce half-payload split point

F32 = mybir.dt.float32
F16 = mybir.dt.float16
U8 = mybir.dt.uint8

AF = mybir.ActivationFunctionType
AX = mybir.AxisListType


def _build_kernel(tc: tile.TileContext, pk: bass.AP, out: bass.AP,
                  stage: int = 7):
    nc = tc.nc

    with (
        tc.tile_pool(name="const", bufs=1) as const_pool,
        tc.tile_pool(name="persist", bufs=1) as persist,
        tc.tile_pool(name="chunks", bufs=NCH) as ch_pool,
        tc.tile_pool(name="dram", bufs=1, space="DRAM") as dram_pool,
        tc.tile_pool(name="smalls", bufs=1) as smalls,
    ):
        # warm up the collective stream before any compute: the runtime's
        # first-collective bootstrap (global barrier + channel setup) then
        # runs concurrently with phase A instead of delaying the stats
        # AllReduce by tens of us.
        warm_in = dram_pool.tile([1, 1], U8)
        warm_out = dram_pool.tile([CORES, 1], U8)
        with tc.high_priority():
            nc.gpsimd.collective_compute(
                "AllGather",
                mybir.AluOpType.bypass,
                replica_groups=[list(range(CORES))],
                ins=[warm_in.opt()],
                outs=[warm_out.opt()],
            )

        ident = const_pool.tile([P, P], F32)
        masks.make_identity(nc, ident[:])
        ident_h = const_pool.tile([P, P], F16)
        nc.scalar.copy(ident_h[:], ident[:])
        ones_h = const_pool.tile([P, 1], F16)
        nc.vector.memset(ones_h[:], 1.0)

        # ---- phase A: stream in packed fp16 shard; accumulate y^T E ----
        mm1_ctx = tc.tile_pool(name="mm1ps", bufs=1, space="PSUM")
        mm1_ps = mm1_ctx.__enter__()
        cent_ps = mm1_ps.tile([P, D], F32)   # 2 banks; halves are the 2 rhs
        cnt_ps = mm1_ps.tile([P, 1], F32)
        pk_tiles = []
        for i in range(NCH):
            t = ch_pool.tile([P, W], F16, tag="pk")
            nc.sync.dma_start(out=t[:], in_=pk[i * P:(i + 1) * P, :])
            pk_tiles.append(t)
            if stage < 2:
                continue
            first, last = (i == 0), (i == NCH - 1)
            y_sl = t[:, D:D + C]
            for h in range(2):
                nc.tensor.matmul(
                    cent_ps[:, h * 512:(h + 1) * 512],
                    lhsT=y_sl,
                    rhs=t[:, h * 512:(h + 1) * 512],
                    start=first, stop=last,
                )
            mm1_last = nc.tensor.matmul(
                cnt_ps[:], lhsT=y_sl, rhs=ones_h[:],
                start=first, stop=last,
            )

        if stage < 2:
            mm1_ctx.__exit__(None, None, None)
            return
        # ---- phase B: AllReduce [C, 1+D] stats in two pipelined halves ----
        # layout: [counts | d-chunks 0..3 | d-chunks 4..7]; half 1 carries
        # the counts so the reciprocal chain starts as soon as it lands.
        stat_sb = persist.tile([P, 1 + D], F32)
        ar_in1 = dram_pool.tile([P, 1 + DH], F32)
        ar_out1 = dram_pool.tile([P, 1 + DH], F32)
        ar_in2 = dram_pool.tile([P, DH], F32)
        ar_out2 = dram_pool.tile([P, DH], F32)
        gcnt = persist.tile([P, 1], F32)
        gcent = [persist.tile([P, P], F32, name=f"gcent{j}") for j in range(DCH)]
        with tc.high_priority():
            nc.scalar.copy(stat_sb[:, 0:1], cnt_ps[:])
            nc.scalar.copy(stat_sb[:, 1:1 + DH], cent_ps[:, 0:DH])
            ar_dma1 = nc.sync.dma_start(out=ar_in1[:],
                                        in_=stat_sb[:, 0:1 + DH])
            if stage >= 3:
                nc.gpsimd.collective_compute(
                    "AllReduce",
                    mybir.AluOpType.add,
                    replica_groups=[list(range(CORES))],
                    ins=[ar_in1.opt()],
                    outs=[ar_out1.opt()],
                )
            # half 2: d-chunks 4..7
            nc.vector.tensor_copy(stat_sb[:, 1 + DH:1 + D], cent_ps[:, DH:D])
            mm1_ctx.__exit__(None, None, None)
            ar_dma2 = nc.sync.dma_start(out=ar_in2[:],
                                        in_=stat_sb[:, 1 + DH:1 + D])
            if stage < 3:
                return
            nc.gpsimd.collective_compute(
                "AllReduce",
                mybir.AluOpType.add,
                replica_groups=[list(range(CORES))],
                ins=[ar_in2.opt()],
                outs=[ar_out2.opt()],
            )
            # counts first so the reciprocal chain starts immediately
            nc.sync.dma_start(out=gcnt[:], in_=ar_out1[:, 0:1])
            for j in range(DCH):
                if j < DCH // 2:
                    src = ar_out1[:, 1 + j * P:1 + (j + 1) * P]
                else:
                    src = ar_out2[:, (j - DCH // 2) * P:(j - DCH // 2 + 1) * P]
                nc.sync.dma_start(out=gcent[j][:], in_=src)

        if stage < 4:
            return
        # ---- phase C: transpose E shard on PE while the AllReduce runs ----
        # j-major so et d-chunk j is complete in mm2 consumption order; 4
        # transposes share one PSUM tile so a single [128,512] copy drains
        # them (PSUM round-trip latency was the phase C bottleneck).
        et = persist.tile([P, DCH * NS], F16)  # d-chunk j at [:, j*NS:(j+1)*NS]
        with tc.tile_pool(name="trps", bufs=4, space="PSUM") as tr_ps:
            k = 0
            for j in range(DCH):
                for i0 in range(0, NCH, 4):
                    tp = tr_ps.tile([P, 512], F16, tag="tr")
                    for q in range(4):
                        tr_inst = nc.tensor.transpose(
                            tp[:, q * P:(q + 1) * P],
                            pk_tiles[i0 + q][:, j * P:(j + 1) * P],
                            ident_h[:])
                        # ordering-only edge (same PE queue): keep transposes
                        # after phase A's matmuls
                        add_dep_helper(tr_inst.ins, mm1_last.ins, sync=False,
                                       reason="transposes after mm1")
                    dst = et[:, j * NS + i0 * P: j * NS + (i0 + 4) * P]
                    if k % 2 == 0:
                        cp_inst = nc.scalar.copy(dst, tp[:])
                    else:
                        cp_inst = nc.vector.tensor_copy(dst, tp[:])
                    # keep the stats chain ahead of the et copies in the
                    # ACT/DVE FIFOs
                    add_dep_helper(cp_inst.ins, ar_dma2.ins, sync=False,
                                   reason="et copies after ar_in dma")
                    k += 1

            if stage < 5:
                return
            # ---- phase C2: cent2 = 2*cent, -sq_c, cent2^T per d-chunk ----
            safe = smalls.tile([P, 1], F32)
            nc.vector.tensor_scalar_max(safe[:], gcnt[:], 1.0)
            r2 = smalls.tile([P, 1], F32)
            nc.vector.reciprocal(r2[:], safe[:])
            nc.vector.tensor_scalar_mul(r2[:], r2[:], 2.0)
            if stage < 5.4:
                return
            sq_tmp = persist.tile([P, D], F32)
            negsq = smalls.tile([P, 1], F32)
            cent2 = [persist.tile([P, P], F16, name=f"cent2_{j}")
                     for j in range(DCH)]
            centT = [persist.tile([P, C], F16, name=f"centT{j}")
                     for j in range(DCH)]
            for j in range(DCH):
                nc.vector.tensor_scalar_mul(cent2[j][:], gcent[j][:], r2[:, 0:1])
                tp = tr_ps.tile([P, 512], F16, tag="tr")
                nc.tensor.transpose(tp[:, 0:P], cent2[j][:], ident_h[:])
                nc.scalar.copy(centT[j][:], tp[:, 0:P])
                # negsq contribution off the critical path (only exp needs it)
                nc.scalar.square(sq_tmp[:, j * P:(j + 1) * P], cent2[j][:])
            nc.vector.reduce_sum(out=negsq[:], in_=sq_tmp[:], axis=AX.X)
            nc.vector.tensor_scalar_mul(negsq[:], negsq[:], -0.25)

        if stage < 6:
            return
        # ---- phase D/E: cross2 = cent2 @ E^T -> exp -> transpose -> softmax
        # b-outer so block b's PSUM completes early and its softmax tail
        # overlaps block b+1's matmuls.
        with (
            tc.tile_pool(name="crossps", bufs=1, space="PSUM") as cross_pool,
            tc.tile_pool(name="tr2ps", bufs=2, space="PSUM") as tr2_ps,
            tc.tile_pool(name="exps", bufs=2) as exp_pool,
            tc.tile_pool(name="outtiles", bufs=2) as out_pool,
            tc.tile_pool(name="sums", bufs=4) as sum_pool,
        ):
            crs = [cross_pool.tile([P, 512], F32, name=f"cr{b}") for b in range(NB)]
            for b in range(NB):
                for j in range(DCH):
                    nc.tensor.matmul(
                        crs[b][:],
                        lhsT=centT[j][:],
                        rhs=et[:, j * NS + b * 512: j * NS + (b + 1) * 512],
                        start=(j == 0), stop=(j == DCH - 1),
                    )
                # exp(cross2 - sq_c) with per-partition bias; [C, 512] fp32
                # (bf16/fp16 exp costs ~7e-3 rel err; fp16 would overflow)
                ex = exp_pool.tile([P, 512], F32, tag="exp")
                nc.scalar.activation(ex[:], crs[b][:], AF.Exp, bias=negsq[:, 0:1],
                                     scale=1.0)
                if stage < 7:
                    continue
                # back to [n, C] orientation; batch the whole 512-row block:
                # 4 transposes -> one PSUM tile, row-sums -> one [128,4]
                # tile, one reciprocal, one output tile, one DMA.
                tp2 = tr2_ps.tile([P, 512], F32, tag="tr2")
                ot = out_pool.tile([P, 4 * C], F32, tag="ot")
                for tt in range(4):
                    nc.tensor.transpose(tp2[:, tt * P:(tt + 1) * P],
                                        ex[:, tt * P:(tt + 1) * P], ident[:])
                    s = sum_pool.tile([P, 1], F32, tag="s")
                    nc.vector.reduce_sum(out=s[:],
                                         in_=tp2[:, tt * P:(tt + 1) * P],
                                         axis=AX.X)
                    rs = sum_pool.tile([P, 1], F32, tag="rs")
                    nc.vector.reciprocal(rs[:], s[:])
                    nc.scalar.activation(ot[:, tt * C:(tt + 1) * C],
                                         tp2[:, tt * P:(tt + 1) * P],
                                         AF.Copy, bias=0.0,
                                         scale=rs[:, 0:1])
                nc.sync.dma_start(
                    out=out[b * 512:(b + 1) * 512, :].rearrange(
                        "(t p) c -> t p c", t=4),
                    in_=ot[:].rearrange("p (t c) -> t p c", t=4))


def build_module(stage: int = 7):
    nc = bacc.Bacc("TRN2", target_bir_lowering=False, debug=False,
                   num_devices=CORES)
    pk = nc.dram_tensor("packed", [NS, W], F16, kind="ExternalInput").ap()
    out = nc.dram_tensor("out", [NS, C], F32, kind="ExternalOutput").ap()
    with tile.TileContext(nc) as tc:
        _build_kernel(tc, pk, out, stage=stage)
    nc.compile()
    return nc


_NC_CACHE = {}


def _get_nc():
    if "nc" not in _NC_CACHE:
        _NC_CACHE["nc"] = build_module()
    return _NC_CACHE["nc"]


def run(embeddings: np.ndarray, y_true: np.ndarray, **spmd_kwargs):
    assert embeddings.shape == (N, D) and y_true.shape == (N, C)
    emb16 = np.asarray(embeddings, dtype=np.float16)
    y16 = np.asarray(y_true, dtype=np.float16)
    packed = np.concatenate([emb16, y16], axis=1)  # [N, W] fp16

    nc = _get_nc()
    in_maps = [
        {"packed": np.ascontiguousarray(packed[k * NS:(k + 1) * NS])}
        for k in range(CORES)
    ]
    res = run_bass_kernel_spmd(nc, in_maps, core_ids=list(range(CORES)),
                               **spmd_kwargs)
    out = np.concatenate([res.results[k]["out"] for k in range(CORES)], axis=0)
    return out, res


def kernel(embeddings: np.ndarray, y_true: np.ndarray) -> np.ndarray:
    out, _ = run(embeddings, y_true)
    return out
